# revision 10
# baseline (speedup 1.0000x reference)
"""DisenTripletGCN on 8 Trainium2 NeuronCores (Bass/Tile, SPMD).

Strategy:
  * Capsule sharding: capsule c of the 8 routing capsules lives on core c.
    The only per-iteration cross-core traffic is an AllReduce(add) of
    exp(logits) (~0.33MB) for the softmax denominator.
  * Degree-sorted node relabeling + slot planes: nodes are relabeled by
    descending target-degree so plane j (the j-th incoming edge of every
    target) aligns with a contiguous prefix of the rank-ordered node state
    in SBUF. The per-iteration u[trg] "gather" becomes dense aligned
    elementwise work; the per-edge z tensor is gathered once per routing
    layer with the hardware dma_gather.
  * f32r (full-rate fp32) matmuls; BN folded into clf weights on host;
    bias via K=1 rank-1 matmul; PE transposes for row-major -> K-major.
  * AllToAll redistributions between node-sharded matmul phases and
    capsule-sharded routing; ReduceScatter for the object pooling.

Self-contained: all static preprocessing (planes, permutations, int16
gather indices, reciprocal counts) is computed from the `edges` input
with numpy and shipped as extra per-core inputs.
"""
import os
import sys

import numpy as np

sys.path.insert(0, "/opt/trn_rl_repo")

import concourse.bacc as bacc
import concourse.mybir as mybir
import concourse.tile as tile
from concourse.bass_utils import run_bass_kernel_spmd
from concourse.masks import make_identity

dt = mybir.dt
Alu = mybir.AluOpType
Act = mybir.ActivationFunctionType

# ---- problem constants -------------------------------------------------
O, T = 20000, 80000
NCAPS, NHID = 8, 64
D = NCAPS * NHID                  # 512
NFEAT1 = 384
NCLASS1 = 768
H, DOUT = 256, 256
ROUTIT = 3
BN_EPS = 1e-5
NCORES = 8

HB = 2560                         # head rows per core (padded node shard)
TB = 7500                         # tail rows per core
ROWS = 10112                      # 79*128, padded per-core edge rows
RT = ROWS // 128                  # 79 row tiles
HT = HB // 128                    # 20 head row tiles
NP = 20096                        # 157*128 rank-padded node count
NCH = NP // 128                   # 157 node chunks
NATR = NCORES * HB                # 20480 natural table rows
T_ROWS = 20608                    # table alloc rows (>= NATR+1)
ZROW = 20480                      # zero row index in every table
JUNK_ROW = 20256                  # junk scatter target (>= O, < NATR)
CB = 32                           # z stream block (chunks)
GCH = 8192                        # gather chunk (indices)

DBG = os.environ.get("KDBG", "0") == "1"
STAGE = int(os.environ.get("KSTAGE", "99"))


# ======================================================================
# Host-side static preprocessing
# ======================================================================

def _wrap_idx(vals: np.ndarray) -> np.ndarray:
    """int16 index layout for dma_gather/scatter_add: [128, n/16]; the
    value for slot i sits at [i%16, i//16], replicated over 8 Q7 stripes."""
    n = vals.shape[0]
    assert n % 16 == 0
    a = np.ascontiguousarray(vals.reshape(n // 16, 16).T.astype(np.int16))
    return np.tile(a, (8, 1))


def build_static(edges: np.ndarray) -> dict:
    s = edges[:, 0].astype(np.int64)
    o = edges[:, 1].astype(np.int64)
    deg = np.bincount(o, minlength=O)
    order = np.argsort(-deg, kind="stable")       # rank -> natural
    rank = np.empty(O, np.int64)
    rank[order] = np.arange(O)

    # slot of each edge within its target's list (grouped by rank(o))
    idx_sorted = np.argsort(rank[o], kind="stable")
    ro = rank[o][idx_sorted]
    first = np.r_[True, ro[1:] != ro[:-1]]
    grp_start = np.maximum.accumulate(np.where(first, np.arange(T), 0))
    slot = np.zeros(T, np.int64)
    slot[idx_sorted] = np.arange(T) - grp_start

    J = int(deg.max())
    n_j = np.array([(deg > j).sum() for j in range(J)])
    n_j_pad = ((n_j + 127) // 128) * 128
    assert n_j_pad.max() <= NP
    off = np.r_[0, np.cumsum(n_j_pad)]
    S = int(off[-1])
    assert S % 128 == 0

    edge_pos = off[slot] + rank[o]
    pos2edge = np.full(S, -1, np.int64)
    pos2edge[edge_pos] = np.arange(T)
    valid = pos2edge >= 0
    src_of_pos = np.where(valid, s[np.clip(pos2edge, 0, T - 1)], 0)

    zidx_nat = np.where(valid, src_of_pos, ZROW)
    zidx_rank = np.where(valid, rank[src_of_pos], ZROW)

    pi = np.full(NP, ZROW, np.int64)
    pi[:O] = order
    clf = np.full(NATR, ZROW, np.int64)
    clf[:O] = rank

    cnt = np.bincount(s, minlength=O) + np.bincount(o, minlength=O)
    rcnt = (1.0 / np.maximum(cnt, 1.0)).astype(np.float32)

    # fixed layout: shard rows [0,HB) = head slots (natural HB*c+r, pad
    # where >= O), rows [HB, HB+TB) = tail, rest pad.
    shards = []
    for c in range(NCORES):
        head = np.arange(HB * c, min(HB * (c + 1), O))
        tail = np.arange(O + TB * c, O + TB * (c + 1))
        shards.append((head, tail))

    plane_chunks = [(int(off[j]) // 128, int(off[j + 1]) // 128) for j in range(J)]

    # scatter waves: per core and per side (s/o), rows grouped so targets
    # are unique within a wave; uniform capacities across cores.
    def side_waves(get_vals):
        per_core = []
        for c in range(NCORES):
            head, tail = shards[c]
            rows_local = np.concatenate([
                np.arange(head.shape[0]),
                np.arange(HB, HB + TB)])
            vals = get_vals(c)
            ordv = np.argsort(vals, kind="stable")
            sv = vals[ordv]
            firstv = np.r_[True, sv[1:] != sv[:-1]]
            gstart = np.maximum.accumulate(
                np.where(firstv, np.arange(sv.shape[0]), 0))
            mult = np.arange(sv.shape[0]) - gstart
            waves = {}
            for pos, w in zip(ordv, mult):
                waves.setdefault(int(w), []).append(int(pos))
            per_core.append((waves, rows_local, vals))
        W = max(max(w.keys()) for w, _, _ in per_core) + 1
        caps = []
        for w in range(W):
            m = max(len(pc[0].get(w, [])) for pc in per_core)
            caps.append(((m + 127) // 128) * 128)
        total = sum(caps)
        gsrc_all, tgt_all = [], []
        for c in range(NCORES):
            waves, rows_local, vals = per_core[c]
            gsrc = np.zeros(total, np.int64)
            tgt = np.zeros(total, np.int64)
            p = 0
            for w in range(W):
                lst = waves.get(w, [])
                npad = caps[w] - len(lst)
                gsrc[p : p + len(lst)] = rows_local[lst]
                tgt[p : p + len(lst)] = vals[lst]
                gsrc[p + len(lst) : p + caps[w]] = 0
                tgt[p + len(lst) : p + caps[w]] = O + (np.arange(npad) % 448)
                p += caps[w]
            gsrc_all.append(gsrc)
            tgt_all.append(tgt)
        return caps, gsrc_all, tgt_all

    def vals_for(side):
        def get(c):
            head, tail = shards[c]
            v = side[np.concatenate([head, tail])]
            return v
        return get

    s_caps, s_gsrc, s_tgt = side_waves(vals_for(s))
    o_caps, o_gsrc, o_tgt = side_waves(vals_for(o))

    return dict(s=s, o=o, order=order, rank=rank, J=J, S=S, SC=S // 128,
                plane_chunks=plane_chunks, zidx_nat=zidx_nat,
                zidx_rank=zidx_rank, pi=pi, clf=clf, rcnt=rcnt, shards=shards,
                s_caps=s_caps, s_gsrc=s_gsrc, s_tgt=s_tgt,
                o_caps=o_caps, o_gsrc=o_gsrc, o_tgt=o_tgt)


def build_inputs(inputs: dict, st: dict) -> list[dict]:
    obj = np.ascontiguousarray(np.asarray(inputs["obj_vecs"], np.float32))
    pred = np.asarray(inputs["pred_vecs"], np.float32)

    g1 = np.asarray(inputs["bn_g1"], np.float32) / np.sqrt(np.float32(1.0 + BN_EPS))
    cw1 = np.asarray(inputs["clf_w1"], np.float32) * g1[None, :]
    cb1 = (np.asarray(inputs["clf_b1"], np.float32) * g1
           + np.asarray(inputs["bn_b1"], np.float32))
    g2 = np.asarray(inputs["bn_g2"], np.float32) / np.sqrt(np.float32(1.0 + BN_EPS))
    cw2 = np.asarray(inputs["clf_w2"], np.float32) * g2[None, :]
    cb2 = (np.asarray(inputs["clf_b2"], np.float32) * g2
           + np.asarray(inputs["bn_b2"], np.float32))

    shared = dict(
        obj_tab=obj,
        w1=np.asarray(inputs["pca_w1"], np.float32),
        b1=np.asarray(inputs["pca_b1"], np.float32).reshape(1, D),
        cw1=np.ascontiguousarray(cw1, np.float32),
        cb1=np.ascontiguousarray(cb1.reshape(1, NCLASS1), np.float32),
        w2=np.asarray(inputs["pca_w2"], np.float32),
        b2=np.asarray(inputs["pca_b2"], np.float32).reshape(1, D),
        cw2=np.ascontiguousarray(cw2, np.float32),
        cb2=np.ascontiguousarray(cb2.reshape(1, DOUT), np.float32),
        zidx1=_wrap_idx(st["zidx_nat"]),
        zidxr=_wrap_idx(st["zidx_rank"]),
        piidx=_wrap_idx(st["pi"]),
        clfidx=_wrap_idx(st["clf"]),
    )

    s, o, rcnt = st["s"], st["o"], st["rcnt"]
    in_maps = []
    for c in range(NCORES):
        head, tail = st["shards"][c]
        nh = head.shape[0]
        pm = np.zeros((ROWS, 128), np.float32)
        pm[:nh] = pred[head]
        pm[HB : HB + TB] = pred[tail]
        sv = np.zeros(ROWS, np.int64)
        ov = np.zeros(ROWS, np.int64)
        sv[:nh] = s[head]
        ov[:nh] = o[head]
        sv[HB : HB + TB] = s[tail]
        ov[HB : HB + TB] = o[tail]

        rc = np.ones(HB, np.float32)
        nn = max(0, min(HB, O - HB * c))
        if nn > 0:
            rc[:nn] = rcnt[HB * c : HB * c + nn]
        rcw = np.empty((128, HB // 128), np.float32)
        lin = np.arange(HB)
        rcw[lin % 128, lin // 128] = rc
        in_maps.append(dict(shared,
                            pred_mine=pm,
                            sidx=_wrap_idx(sv),
                            oidx=_wrap_idx(ov),
                            swv_g=_wrap_idx(st["s_gsrc"][c]),
                            swv_t=_wrap_idx(st["s_tgt"][c]),
                            owv_g=_wrap_idx(st["o_gsrc"][c]),
                            owv_t=_wrap_idx(st["o_tgt"][c]),
                            rcnt_w=rcw,
                            ones_in=np.ones((1, 128), np.float32)))
    return in_maps


# ======================================================================
# Device program
# ======================================================================

def build_nc(st: dict):
    S, SC = st["S"], st["SC"]
    plane_chunks = st["plane_chunks"]

    nc = bacc.Bacc("TRN2", target_bir_lowering=False, debug=False,
                   num_devices=NCORES)

    def din(name, shape, d=dt.float32):
        return nc.declare_dram_parameter(name, list(shape), d, isOutput=False)

    def dout(name, shape, d=dt.float32):
        return nc.declare_dram_parameter(name, list(shape), d, isOutput=True)

    obj_tab = din("obj_tab", [O, 128])
    pred_mine = din("pred_mine", [ROWS, 128])
    w1 = din("w1", [NFEAT1, D])
    b1 = din("b1", [1, D])
    cw1 = din("cw1", [D, NCLASS1])
    cb1 = din("cb1", [1, NCLASS1])
    w2 = din("w2", [H, D])
    b2 = din("b2", [1, D])
    cw2 = din("cw2", [D, DOUT])
    cb2 = din("cb2", [1, DOUT])
    zidx1 = din("zidx1", [128, S // 16], dt.int16)
    zidxr = din("zidxr", [128, S // 16], dt.int16)
    piidx = din("piidx", [128, NP // 16], dt.int16)
    clfidx = din("clfidx", [128, NATR // 16], dt.int16)
    sidx = din("sidx", [128, ROWS // 16], dt.int16)
    oidx = din("oidx", [128, ROWS // 16], dt.int16)
    SWT = sum(st["s_caps"])
    OWT = sum(st["o_caps"])
    swv_g = din("swv_g", [128, SWT // 16], dt.int16)
    swv_t = din("swv_t", [128, SWT // 16], dt.int16)
    owv_g = din("owv_g", [128, OWT // 16], dt.int16)
    owv_t = din("owv_t", [128, OWT // 16], dt.int16)
    rcnt_w = din("rcnt_w", [128, HB // 128])
    ones_in = din("ones_in", [1, 128])

    obj_out_sh = dout("obj_out_sh", [HB, DOUT])
    pred_out_sh = dout("pred_out_sh", [ROWS, H])

    tableA = nc.dram_tensor("tableA", [T_ROWS, NHID], dt.float32)
    tableB = nc.dram_tensor("tableB", [T_ROWS, NHID], dt.float32)
    z_dram = nc.dram_tensor("z_dram", [128, SC, NHID], dt.float32)
    x3t_dram = nc.dram_tensor("x3t_dram", [128, RT, D], dt.float32)
    ns_dram = nc.dram_tensor("ns_dram", [ROWS, H], dt.float32)
    no_dram = nc.dram_tensor("no_dram", [ROWS, H], dt.float32)
    pooled_dram = nc.dram_tensor("pooled_dram", [NATR, H], dt.float32)
    a2a_in = nc.dram_tensor("a2a_in", [NCORES * HB, NHID], dt.float32)
    a2a_out = nc.dram_tensor("a2a_out", [NCORES * HB, NHID], dt.float32)
    cc_in = nc.dram_tensor("cc_in", [128, SC], dt.float32)
    cc_out = nc.dram_tensor("cc_out", [128, SC], dt.float32)
    rs_out = nc.dram_tensor("rs_out", [HB, H], dt.float32)

    if DBG:
        dbg_x3 = dout("dbg_x3", [128, RT, D])
        dbg_tab = dout("dbg_tab", [NATR, NHID])
        dbg_u = dout("dbg_u", [128, NCH, NHID])
        dbg_x2h = dout("dbg_x2h", [NCORES * HB, NHID])
        dbg_pool = dout("dbg_pool", [HB, H])

    RG = [list(range(NCORES))]

    with tile.TileContext(nc) as tc:
        with tc.tile_pool(name="wp", bufs=1) as wp:
            ident = wp.tile([128, 128], dt.float32)
            make_identity(nc, ident[:])
            ones1 = wp.tile([1, 128], dt.float32r)
            nc.sync.dma_start(ones1[:], ones_in[:].bitcast(dt.float32r))

            # ---------- small helpers ----------
            def load_f32r(pool, src, k_tiles, n):
                t = pool.tile([128, k_tiles, n], dt.float32r)
                for k in range(k_tiles):
                    nc.sync.dma_start(
                        t[:, k], src[128 * k : 128 * (k + 1), :].bitcast(dt.float32r))
                return t

            def load_idx(pool, src, tag=None):
                t = pool.tile([128, src.shape[1]], dt.int16,
                              **({"tag": tag} if tag else {}))
                nc.sync.dma_start(t[:], src[:])
                return t

            def transpose_tile(psp, sbp, src_ap):
                pt = psp.tile([128, 128], dt.float32, tag="tposep")
                nc.tensor.transpose(pt[:], src_ap, ident[:])
                ot = sbp.tile([128, 128], dt.float32r, tag="tposeo")
                nc.vector.tensor_copy(ot[:], pt[:])
                return ot

            def l2norm_rowtile(sbp, x, out=None):
                sq = sbp.tile([128, D], dt.float32, tag="l2sq")
                nc.scalar.activation(sq[:], x, Act.Square)
                nrm = sbp.tile([128, NCAPS], dt.float32, tag="l2n")
                nc.vector.tensor_reduce(
                    nrm[:], sq[:].rearrange("p (k h) -> p k h", k=NCAPS),
                    mybir.AxisListType.X, Alu.add)
                nc.scalar.activation(nrm[:], nrm[:], Act.Sqrt)
                nc.vector.tensor_scalar_max(nrm[:], nrm[:], 1e-12)
                nc.vector.reciprocal(nrm[:], nrm[:])
                if out is None:
                    out = sbp.tile([128, D], dt.float32, tag="l2o")
                rb = nrm[:].rearrange("p (k o) -> p k o", o=1).broadcast_to(
                    [128, NCAPS, NHID])
                nc.vector.tensor_tensor(
                    out[:].rearrange("p (k h) -> p k h", k=NCAPS),
                    x.rearrange("p (k h) -> p k h", k=NCAPS), rb, Alu.mult)
                return out

            def zero_table_row(pool, table):
                z = pool.tile([1, NHID], dt.float32, tag="zrow")
                nc.vector.memset(z[:], 0.0)
                nc.sync.dma_start(table.ap()[ZROW : ZROW + 1, :], z[:])

            # ==========================================================
            # Stage 1: pca1 -> x3 rows; head capsule slices -> a2a_in
            # ==========================================================
            with (
                tc.tile_pool(name="s1g", bufs=1) as s1g,
                tc.tile_pool(name="s1", bufs=3) as s1,
                tc.tile_pool(name="s1ps", bufs=3, space="PSUM") as s1ps,
            ):
                sidx_t = load_idx(s1g, sidx)
                oidx_t = load_idx(s1g, oidx)
                gsb = s1g.tile([128, RT, 128], dt.float32)
                gob = s1g.tile([128, RT, 128], dt.float32)
                prb = s1g.tile([128, RT, 128], dt.float32)
                for g0 in range(0, ROWS, GCH):
                    n = min(GCH, ROWS - g0)
                    nc.gpsimd.dma_gather(
                        gsb[:, g0 // 128 : (g0 + n) // 128], obj_tab[:],
                        sidx_t[:, g0 // 16 : (g0 + n) // 16], n, n, 128,
                        single_packet=False)
                    nc.gpsimd.dma_gather(
                        gob[:, g0 // 128 : (g0 + n) // 128], obj_tab[:],
                        oidx_t[:, g0 // 16 : (g0 + n) // 16], n, n, 128,
                        single_packet=False)
                nc.sync.dma_start(
                    prb[:], pred_mine[:].rearrange("(q p) d -> p q d", p=128))

                w1s = load_f32r(s1g, w1, 3, D)
                b1s = s1g.tile([1, D], dt.float32r)
                nc.sync.dma_start(b1s[:], b1[:].bitcast(dt.float32r))

                for rt in range(RT):
                    tS = transpose_tile(s1ps, s1, gsb[:, rt])
                    tP = transpose_tile(s1ps, s1, prb[:, rt])
                    tO = transpose_tile(s1ps, s1, gob[:, rt])
                    ps = s1ps.tile([128, D], dt.float32, tag="mm")
                    nc.tensor.matmul(ps[:], tS[:], w1s[:, 0], start=True, stop=False)
                    nc.tensor.matmul(ps[:], tP[:], w1s[:, 1], start=False, stop=False)
                    nc.tensor.matmul(ps[:], tO[:], w1s[:, 2], start=False, stop=False)
                    nc.tensor.matmul(ps[:], ones1[:], b1s[:], start=False, stop=True)
                    x1 = s1.tile([128, D], dt.float32, tag="x1")
                    nc.scalar.activation(x1[:], ps[:], Act.Lrelu, alpha=0.01)
                    x3 = l2norm_rowtile(s1, x1[:])
                    nc.sync.dma_start(x3t_dram.ap()[:, rt], x3[:])
                    if rt < HT:
                        for k in range(NCAPS):
                            nc.sync.dma_start(
                                a2a_in.ap()[HB * k + 128 * rt : HB * k + 128 * (rt + 1), :],
                                x3[:, NHID * k : NHID * (k + 1)])
                if DBG:
                    for rt in range(RT):
                        dtile = s1.tile([128, D], dt.float32, tag="l2sq")
                        nc.sync.dma_start(dtile[:], x3t_dram.ap()[:, rt])
                        nc.sync.dma_start(dbg_x3.ap()[:, rt], dtile[:])

            # ==========================================================
            # routing machinery (state allocated per routing stage)
            # ==========================================================
            def make_routing_state(pool):
                return dict(
                    U=pool.tile([128, NCH, NHID], dt.float32, name="rsU"),
                    X3H=pool.tile([128, NCH, NHID], dt.float32, name="rsX3H"),
                    L=pool.tile([128, SC], dt.float32, name="rsL"),
                    E=pool.tile([128, SC], dt.float32, name="rsE"),
                    P=pool.tile([128, SC], dt.float32, name="rsP"),
                    n2=pool.tile([128, NCH], dt.float32, name="rsn2"),
                    rsc=pool.tile([128, NCH], dt.float32, name="rsrsc"),
                )

            def gather_into(dst_view, table, idx_t, total, elem):
                for g0 in range(0, total, GCH):
                    n = min(GCH, total - g0)
                    nc.gpsimd.dma_gather(
                        dst_view[:, g0 // 128 : (g0 + n) // 128], table.ap()[:, :],
                        idx_t[:, g0 // 16 : (g0 + n) // 16], n, n, elem,
                        single_packet=False)

            def write_rank_table(table, src_tile):
                nc.sync.dma_start(
                    table.ap()[:NP, :].rearrange("(q p) d -> p q d", p=128),
                    src_tile[:])

            def normalize_into(rs, zpool, src, dst):
                """dst = per-capsule l2norm(src) over the node state."""
                for q0 in range(0, NCH, CB):
                    n = min(CB, NCH - q0)
                    blk = zpool.tile([128, CB, NHID], dt.float32, tag="mt")
                    nc.scalar.activation(blk[:, :n], src[:, q0 : q0 + n], Act.Square)
                    nc.vector.tensor_reduce(
                        rs["n2"][:, q0 : q0 + n], blk[:, :n],
                        mybir.AxisListType.X, Alu.add)
                nc.scalar.activation(rs["n2"][:], rs["n2"][:], Act.Sqrt)
                nc.vector.tensor_scalar_max(rs["n2"][:], rs["n2"][:], 1e-12)
                nc.vector.reciprocal(rs["rsc"][:], rs["n2"][:])
                rb = rs["rsc"][:].rearrange("p (q o) -> p q o", o=1).broadcast_to(
                    [128, NCH, NHID])
                nc.vector.tensor_tensor(dst[:], src[:], rb, Alu.mult)

            def routing_layer(rs, zpool, table, idx_t):
                U, X3H = rs["U"], rs["X3H"]
                L, E, P = rs["L"], rs["E"], rs["P"]
                # z planes gather -> z_dram
                for g0 in range(0, S, GCH):
                    n = min(GCH, S - g0)
                    zt = zpool.tile([128, GCH // 128, NHID], dt.float32, tag="zg")
                    nc.gpsimd.dma_gather(
                        zt[:, : n // 128], table.ap()[:, :],
                        idx_t[:, g0 // 16 : (g0 + n) // 16], n, n, NHID,
                        single_packet=False)
                    nc.sync.dma_start(
                        z_dram.ap()[:, g0 // 128 : (g0 + n) // 128, :],
                        zt[:, : n // 128])

                for _ in range(ROUTIT):
                    # A: logits
                    for (c0, c1) in plane_chunks:
                        for b0 in range(c0, c1, CB):
                            n = min(CB, c1 - b0)
                            q0 = b0 - c0
                            zb = zpool.tile([128, CB, NHID], dt.float32, tag="zs")
                            nc.sync.dma_start(
                                zb[:, :n], z_dram.ap()[:, b0 : b0 + n, :])
                            mt = zpool.tile([128, CB, NHID], dt.float32, tag="mt")
                            nc.vector.tensor_tensor(
                                mt[:, :n], zb[:, :n], U[:, q0 : q0 + n], Alu.mult)
                            nc.vector.tensor_reduce(
                                L[:, b0 : b0 + n], mt[:, :n],
                                mybir.AxisListType.X, Alu.add)
                    # B: softmax denominator (TAU = 1)
                    nc.scalar.activation(E[:], L[:], Act.Exp)
                    nc.sync.dma_start(cc_in.ap()[:, :], E[:])
                    nc.gpsimd.collective_compute(
                        "AllReduce", Alu.add, replica_groups=RG,
                        ins=[cc_in.ap().opt()], outs=[cc_out.ap().opt()])
                    nc.sync.dma_start(P[:], cc_out.ap()[:, :])
                    nc.vector.reciprocal(P[:], P[:])
                    nc.vector.tensor_tensor(P[:], E[:], P[:], Alu.mult)
                    # C: U <- x3 + sum_j p_j * z_j (U free after phase A)
                    nc.vector.tensor_copy(U[:], X3H[:])
                    for (c0, c1) in plane_chunks:
                        for b0 in range(c0, c1, CB):
                            n = min(CB, c1 - b0)
                            q0 = b0 - c0
                            zb = zpool.tile([128, CB, NHID], dt.float32, tag="zs")
                            nc.sync.dma_start(
                                zb[:, :n], z_dram.ap()[:, b0 : b0 + n, :])
                            wt = zpool.tile([128, CB, NHID], dt.float32, tag="mt")
                            pb = P[:, b0 : b0 + n].rearrange(
                                "p (c o) -> p c o", o=1).broadcast_to(
                                [128, n, NHID])
                            nc.vector.tensor_tensor(wt[:, :n], zb[:, :n], pb, Alu.mult)
                            nc.vector.tensor_tensor(
                                U[:, q0 : q0 + n], U[:, q0 : q0 + n],
                                wt[:, :n], Alu.add)
                    # D: U = l2norm(U)
                    normalize_into(rs, zpool, U, U)

            def routing_net(dbg_u_out=None):
                """AllToAll (a2a_in prefilled) -> tableA -> 2 routing layers
                -> u2 in tableB (rank order)."""
                with (
                    tc.tile_pool(name="rt", bufs=1) as rtp,
                    tc.tile_pool(name="rtz", bufs=2) as rtz,
                ):
                    nc.gpsimd.collective_compute(
                        "AllToAll", Alu.bypass, replica_groups=RG,
                        ins=[a2a_in.ap().opt()], outs=[a2a_out.ap().opt()])
                    nc.sync.dma_start(tableA.ap()[:NATR, :], a2a_out.ap()[:, :])
                    zero_table_row(rtp, tableA)
                    if DBG and dbg_u_out is not None:
                        nc.sync.dma_start(dbg_tab.ap()[:, :], tableA.ap()[:NATR, :])

                    rs = make_routing_state(rtp)
                    with tc.tile_pool(name="rti", bufs=1) as rti:
                        piidx_t = load_idx(rti, piidx)
                        gather_into(rs["U"][:], tableA, piidx_t, NP, NHID)
                    nc.vector.tensor_copy(rs["X3H"][:], rs["U"][:])
                    with tc.tile_pool(name="rtx1", bufs=1) as rtx:
                        zidx1_t = load_idx(rtx, zidx1)
                        routing_layer(rs, rtz, tableA, zidx1_t)
                    if DBG and dbg_u_out is not None:
                        nc.sync.dma_start(dbg_u_out.ap()[:, :, :], rs["U"][:])
                    # layer 2
                    normalize_into(rs, rtz, rs["U"], rs["U"])
                    zero_table_row(rtp, tableB)
                    write_rank_table(tableB, rs["U"])
                    nc.vector.tensor_copy(rs["X3H"][:], rs["U"][:])
                    with tc.tile_pool(name="rtx2", bufs=1) as rtx:
                        zidxr_t = load_idx(rtx, zidxr)
                        routing_layer(rs, rtz, tableB, zidxr_t)
                    write_rank_table(tableB, rs["U"])

            def clf_redistribute():
                """tableB (rank u2) -> AllToAll -> a2a_out natural blocks."""
                with tc.tile_pool(name="cg", bufs=2) as cg:
                    clfidx_t = load_idx(cg, clfidx)
                    for g0 in range(0, NATR, GCH):
                        n = min(GCH, NATR - g0)
                        gt = cg.tile([128, GCH // 128, NHID], dt.float32, tag="gt")
                        nc.gpsimd.dma_gather(
                            gt[:, : n // 128], tableB.ap()[:, :],
                            clfidx_t[:, g0 // 16 : (g0 + n) // 16], n, n, NHID,
                        single_packet=False)
                        nc.sync.dma_start(
                            a2a_in.ap()[g0 : g0 + n, :].rearrange(
                                "(q p) d -> p q d", p=128),
                            gt[:, : n // 128])
                    nc.gpsimd.collective_compute(
                        "AllToAll", Alu.bypass, replica_groups=RG,
                        ins=[a2a_in.ap().opt()], outs=[a2a_out.ap().opt()])

            # ==========================================================
            # net1 routing
            # ==========================================================
            if STAGE >= 2:
                routing_net(dbg_u if DBG else None)

            # ==========================================================
            # clf1 + pooling
            # ==========================================================
            if STAGE >= 3:
              clf_redistribute()
              if DBG:
                nc.sync.dma_start(dbg_x2h.ap()[:, :], a2a_out.ap()[:, :])

              with (
                tc.tile_pool(name="c1", bufs=3) as c1,
                tc.tile_pool(name="c1g", bufs=1) as c1g,
                tc.tile_pool(name="c1ps", bufs=3, space="PSUM") as c1ps,
            ):
                cw1s = load_f32r(c1g, cw1, 4, NCLASS1)
                cb1s = c1g.tile([1, NCLASS1], dt.float32r)
                nc.sync.dma_start(cb1s[:], cb1[:].bitcast(dt.float32r))

                for rt in range(RT):
                    xin = c1.tile([128, D], dt.float32, tag="xin")
                    if rt < HT:
                        for k in range(NCAPS):
                            nc.sync.dma_start(
                                xin[:, NHID * k : NHID * (k + 1)],
                                a2a_out.ap()[HB * k + 128 * rt : HB * k + 128 * (rt + 1), :])
                        xv = xin
                    else:
                        nc.sync.dma_start(xin[:], x3t_dram.ap()[:, rt])
                        xv = l2norm_rowtile(c1, xin[:])
                    kt = [transpose_tile(c1ps, c1, xv[:, 128 * k : 128 * (k + 1)])
                          for k in range(4)]
                    o1 = c1.tile([128, NCLASS1], dt.float32, tag="o1")
                    for nb in range(2):
                        ps = c1ps.tile([128, 384], dt.float32, tag="mmc")
                        for k in range(4):
                            nc.tensor.matmul(
                                ps[:], kt[k][:],
                                cw1s[:, k, 384 * nb : 384 * (nb + 1)],
                                start=(k == 0), stop=False)
                        nc.tensor.matmul(ps[:], ones1[:],
                                         cb1s[:, 384 * nb : 384 * (nb + 1)],
                                         start=False, stop=True)
                        nc.scalar.activation(o1[:, 384 * nb : 384 * (nb + 1)],
                                             ps[:], Act.Lrelu, alpha=0.01)
                    nc.sync.dma_start(
                        pred_out_sh.ap()[128 * rt : 128 * (rt + 1), :],
                        o1[:, H : 2 * H])
                    nc.sync.dma_start(
                        ns_dram.ap()[128 * rt : 128 * (rt + 1), :], o1[:, :H])
                    nc.sync.dma_start(
                        no_dram.ap()[128 * rt : 128 * (rt + 1), :], o1[:, 2 * H :])

            with tc.tile_pool(name="pz", bufs=2) as pz:
                zt = pz.tile([128, 10, H], dt.float32)
                nc.vector.memset(zt[:], 0.0)
                for b in range(0, NATR, 1280):
                    nc.sync.dma_start(
                        pooled_dram.ap()[b : b + 1280, :].rearrange(
                            "(q p) d -> p q d", p=128), zt[:])
                for src_d, gsrc_in, tgt_in, caps in (
                    (ns_dram, swv_g, swv_t, st["s_caps"]),
                    (no_dram, owv_g, owv_t, st["o_caps"]),
                ):
                    g_t = load_idx(pz, gsrc_in, tag="wgi")
                    t_t = load_idx(pz, tgt_in, tag="wti")
                    p0 = 0
                    for cap in caps:
                        for g0 in range(0, cap, 2048):
                            n = min(2048, cap - g0)
                            buf = pz.tile([128, 16, H], dt.float32,
                                          name="wbuf", tag="wbuf")
                            nc.gpsimd.dma_gather(
                                buf[:, : n // 128], src_d.ap()[:, :],
                                g_t[:, (p0 + g0) // 16 : (p0 + g0 + n) // 16],
                                n, n, H, single_packet=False)
                            nc.gpsimd.dma_scatter_add(
                                pooled_dram.ap()[:, :], buf[:, : n // 128],
                                t_t[:, (p0 + g0) // 16 : (p0 + g0 + n) // 16],
                                n, n, H, single_packet=False)
                        p0 += cap
                nc.gpsimd.collective_compute(
                    "ReduceScatter", Alu.add, replica_groups=RG,
                    ins=[pooled_dram.ap().opt()], outs=[rs_out.ap().opt()])

            # ==========================================================
            # net2: pca2 -> a2a_in
            # ==========================================================
            with (
                tc.tile_pool(name="n2", bufs=3) as n2p,
                tc.tile_pool(name="n2g", bufs=1) as n2g,
                tc.tile_pool(name="n2ps", bufs=3, space="PSUM") as n2ps,
            ):
                pb = n2g.tile([128, HB // 128, H], dt.float32)
                nc.sync.dma_start(
                    pb[:], rs_out.ap()[:, :].rearrange("(q p) d -> p q d", p=128))
                rcw = n2g.tile([128, HB // 128], dt.float32)
                nc.sync.dma_start(rcw[:], rcnt_w[:])
                rbb = rcw[:].rearrange("p (q o) -> p q o", o=1).broadcast_to(
                    [128, HB // 128, H])
                nc.vector.tensor_tensor(pb[:], pb[:], rbb, Alu.mult)
                if DBG:
                    nc.sync.dma_start(
                        dbg_pool.ap()[:, :].rearrange("(q p) d -> p q d", p=128),
                        pb[:])

                w2s = load_f32r(n2g, w2, 2, D)
                b2s = n2g.tile([1, D], dt.float32r)
                nc.sync.dma_start(b2s[:], b2[:].bitcast(dt.float32r))

                for rt in range(HT):
                    t0 = transpose_tile(n2ps, n2p, pb[:, rt, 0:128])
                    t1 = transpose_tile(n2ps, n2p, pb[:, rt, 128:256])
                    ps = n2ps.tile([128, D], dt.float32, tag="mm2")
                    nc.tensor.matmul(ps[:], t0[:], w2s[:, 0], start=True, stop=False)
                    nc.tensor.matmul(ps[:], t1[:], w2s[:, 1], start=False, stop=False)
                    nc.tensor.matmul(ps[:], ones1[:], b2s[:], start=False, stop=True)
                    xb = n2p.tile([128, D], dt.float32, tag="xb")
                    nc.scalar.activation(xb[:], ps[:], Act.Lrelu, alpha=0.01)
                    x3n = l2norm_rowtile(n2p, xb[:])
                    for k in range(NCAPS):
                        nc.sync.dma_start(
                            a2a_in.ap()[HB * k + 128 * rt : HB * k + 128 * (rt + 1), :],
                            x3n[:, NHID * k : NHID * (k + 1)])

            # net2 routing + clf2
            routing_net(None)
            clf_redistribute()

            with (
                tc.tile_pool(name="c2", bufs=3) as c2,
                tc.tile_pool(name="c2g", bufs=1) as c2g,
                tc.tile_pool(name="c2ps", bufs=3, space="PSUM") as c2ps,
            ):
                cw2s = load_f32r(c2g, cw2, 4, DOUT)
                cb2s = c2g.tile([1, DOUT], dt.float32r)
                nc.sync.dma_start(cb2s[:], cb2[:].bitcast(dt.float32r))

                for rt in range(HT):
                    xin = c2.tile([128, D], dt.float32, tag="xin2")
                    for k in range(NCAPS):
                        nc.sync.dma_start(
                            xin[:, NHID * k : NHID * (k + 1)],
                            a2a_out.ap()[HB * k + 128 * rt : HB * k + 128 * (rt + 1), :])
                    kt = [transpose_tile(c2ps, c2, xin[:, 128 * k : 128 * (k + 1)])
                          for k in range(4)]
                    ps = c2ps.tile([128, DOUT], dt.float32, tag="mmo")
                    for k in range(4):
                        nc.tensor.matmul(ps[:], kt[k][:], cw2s[:, k],
                                         start=(k == 0), stop=False)
                    nc.tensor.matmul(ps[:], ones1[:], cb2s[:], start=False, stop=True)
                    oo = c2.tile([128, DOUT], dt.float32, tag="oo")
                    nc.scalar.activation(oo[:], ps[:], Act.Lrelu, alpha=0.01)
                    nc.sync.dma_start(
                        obj_out_sh.ap()[128 * rt : 128 * (rt + 1), :], oo[:])

    nc.compile()
    return nc


# ======================================================================
# Entry point
# ======================================================================

_CACHE: dict = {}


def kernel(**inputs) -> tuple:
    edges = np.asarray(inputs["edges"])
    st = build_static(edges)

    key = ("nc", st["S"], st["J"])
    if key not in _CACHE:
        _CACHE[key] = build_nc(st)
    nc = _CACHE[key]

    in_maps = build_inputs(inputs, st)
    res = run_bass_kernel_spmd(nc, in_maps, list(range(NCORES))).results

    obj_out = np.empty((O, DOUT), np.float32)
    pred_out = np.empty((T, H), np.float32)
    for c in range(NCORES):
        head, tail = st["shards"][c]
        nvalid = head.shape[0]
        obj_out[head] = res[c]["obj_out_sh"][:nvalid]
        po = res[c]["pred_out_sh"]
        pred_out[head] = po[:nvalid]
        pred_out[tail] = po[HB : HB + TB]
    return obj_out, pred_out
